# revision 4
# baseline (speedup 1.0000x reference)
"""Distributed Trainium2 Bass kernel for a single causal attention head.

Problem: x [4, 4096, 1024] f32, Wq/Wk/Wv [64, 1024] f32.
  q/k/v = x @ W.T ; scores = q k^T / sqrt(64); causal softmax; out = attn @ v.
Output: [4, 4096, 64] f32.

Sharding (8 cores): one batch per core-pair; within a pair the sequence is
interleaved at 512-row granularity (even core: 512-blocks {0,2,4,6}, odd:
{1,3,5,7}).  Each core computes Q/K/V for its own 2048 rows from a
host-pre-transposed x^T shard, the pair exchanges [K^T;V^T] via a pairwise
AllGather, then each core runs block-causal attention for its queries over
all 4096 keys.

SPMD: all 8 cores execute one graph.  The causal structure differs between
pair members, so the schedule processes, for q-subtile j (512 queries),
the union count U_j = 8j+8 key chunks (128 keys each); the last 8 slots of
each subtile multiply by a per-core mask *datum* (even core: diagonal
masks then zeros; odd core: ones then diagonal masks).  80 chunk slots per
core vs 72 useful -- 11% padding buys graph uniformity.

Layouts:
 - x^T [1024, 2048] f32 uploaded pre-transposed (projections contract E on
   partitions; no on-chip transpose).  Matmuls use dtype float32r -- the
   full-rate fp32 TensorE path for moving dim >= 256.
 - Projections: stationary [Wk^T|Wv^T] and [Wq^T|Wq^T] e-chunks ->
   [K^T;V^T] and [Q^T;Q^T] tiles [128, 512] accumulated over 8 e-chunks.
 - scoresT [k, q]: stationary K^T [64, 128], moving Q^T [64, 512].  exp
   comes straight off PSUM via ScalarE with the 1/sqrt(H) scale fused; no
   max-subtraction (scores ~ N(0,1), exp overflow impossible).
 - PV: stationary V' = [V | 1] [128, 65] (ones column yields the softmax
   denominators), moving p^T bf16, accumulating out^T [65, 512] in PSUM.
 - Finish: transpose out^T -> [128, 65], reciprocal of col 64, scale.
"""

import numpy as np
import ml_dtypes

import concourse.bacc as bacc
import concourse.mybir as mybir
import concourse.tile as tile
from concourse.bass_utils import run_bass_kernel_spmd

B, S, E, H = 4, 4096, 1024, 64
NCORES = 8
LR = 2048            # local rows per core
QT = 512             # q tile width
KC = 128             # key chunk
NQT = LR // QT       # 4 q-subtiles per core
NEC = E // 128       # 8 e-chunks
GROUPS = [[0, 1], [2, 3], [4, 5], [6, 7]]

BF = mybir.dt.bfloat16
F32 = mybir.dt.float32
F32R = mybir.dt.float32r

_NC_CACHE = {}


def _gid(g):
    """Global 128-key chunk g (0..31) -> gathered chunk index.

    Gathered key order after the pair AllGather is [even core rows; odd
    core rows]; each core's rows are its 512-blocks in ascending order.
    """
    blk = g // 4           # global 512-block 0..7
    w = g % 4
    if blk % 2 == 0:
        return (blk // 2) * 4 + w
    return 16 + (blk // 2) * 4 + w


def build_nc():
    if "nc" in _NC_CACHE:
        return _NC_CACHE["nc"]

    nc = bacc.Bacc("TRN2", target_bir_lowering=False, debug=False,
                   num_devices=NCORES)

    xt_ext = nc.dram_tensor("xt", [E, LR], F32R, kind="ExternalInput")
    wkv_ext = nc.dram_tensor("wkv", [E, 128], F32R, kind="ExternalInput")
    wqq_ext = nc.dram_tensor("wqq", [E, 128], F32R, kind="ExternalInput")
    masks_ext = nc.dram_tensor("masks", [8, 128, QT], BF, kind="ExternalInput")
    idb_ext = nc.dram_tensor("idb", [128, 128], BF, kind="ExternalInput")
    idf_ext = nc.dram_tensor("idf", [128, 128], F32, kind="ExternalInput")
    out_ext = nc.dram_tensor("out", [LR, H], F32, kind="ExternalOutput")

    with tile.TileContext(nc) as tc:
        with (
            tc.tile_pool(name="xt", bufs=NEC) as xt_pool,
            tc.tile_pool(name="const", bufs=1) as const_pool,
            tc.tile_pool(name="proj", bufs=1) as proj_pool,
            tc.tile_pool(name="gath", bufs=1) as gath_pool,
            tc.tile_pool(name="work", bufs=3) as work_pool,
            tc.tile_pool(name="outw", bufs=3) as out_pool,
            tc.tile_pool(name="pkvq", bufs=1, space="PSUM") as pkvq_pool,
            tc.tile_pool(name="pvt", bufs=1, space="PSUM") as pvt_pool,
            tc.tile_pool(name="psc", bufs=2, space="PSUM") as psc_pool,
            tc.tile_pool(name="pout", bufs=2, space="PSUM") as pout_pool,
            tc.tile_pool(name="pops", bufs=1, space="PSUM") as pops_pool,
            tc.tile_pool(name="dram", bufs=1, space="DRAM") as dram_pool,
        ):
            # ---- constants ----
            wkv_sb = const_pool.tile([128, NEC, 128], F32R, tag="wkv")
            nc.sync.dma_start(out=wkv_sb[:], in_=wkv_ext.ap().rearrange(
                "(c p) m -> p c m", p=128))
            wqq_sb = const_pool.tile([128, NEC, 128], F32R, tag="wqq")
            nc.sync.dma_start(out=wqq_sb[:], in_=wqq_ext.ap().rearrange(
                "(c p) m -> p c m", p=128))
            masks_sb = const_pool.tile([128, 8, QT], BF, tag="masks")
            nc.sync.dma_start(out=masks_sb[:], in_=masks_ext.ap().rearrange(
                "m p q -> p m q"))
            idb_sb = const_pool.tile([128, 128], BF, tag="idb")
            nc.sync.dma_start(out=idb_sb[:], in_=idb_ext[:])
            idf_sb = const_pool.tile([128, 128], F32, tag="idf")
            nc.sync.dma_start(out=idf_sb[:], in_=idf_ext[:])

            # ---- load x^T ----
            xt_sb = []
            for ec in range(NEC):
                t = xt_pool.tile([128, LR], F32R, tag="xt")
                nc.sync.dma_start(out=t[:], in_=xt_ext[ec * 128:(ec + 1) * 128, :])
                xt_sb.append(t)

            # ---- QKV projections ----
            kvt_sb = proj_pool.tile([128, LR], BF, tag="kvt")   # [K^T; V^T]
            qqt_sb = proj_pool.tile([128, LR], BF, tag="qqt")   # [Q^T; Q^T]
            for qt in range(NQT):
                qs = slice(qt * QT, (qt + 1) * QT)
                kv_ps = pkvq_pool.tile([128, QT], F32, tag="kvps")
                for ec in range(NEC):
                    nc.tensor.matmul(kv_ps[:], wkv_sb[:, ec, :],
                                     xt_sb[ec][:, qs],
                                     start=(ec == 0), stop=(ec == NEC - 1))
                nc.vector.tensor_copy(kvt_sb[:, qs], kv_ps[:])
                qq_ps = pkvq_pool.tile([128, QT], F32, tag="qqps")
                for ec in range(NEC):
                    nc.tensor.matmul(qq_ps[:], wqq_sb[:, ec, :],
                                     xt_sb[ec][:, qs],
                                     start=(ec == 0), stop=(ec == NEC - 1))
                nc.vector.tensor_copy(qqt_sb[:, qs], qq_ps[:])

            # ---- pairwise K/V exchange ----
            kv_bounce = dram_pool.tile([128, LR], BF, tag="kvb")
            kv_gath = dram_pool.tile([256, LR], BF, tag="kvg")
            nc.sync.dma_start(out=kv_bounce[:], in_=kvt_sb[:])
            nc.gpsimd.collective_compute(
                "AllGather",
                mybir.AluOpType.bypass,
                replica_groups=GROUPS,
                ins=[kv_bounce[:]],
                outs=[kv_gath[:]],
            )
            kt_sb = gath_pool.tile([64, S], BF, tag="kt")
            vt_sb = gath_pool.tile([64, S], BF, tag="vt")
            nc.sync.dma_start(out=kt_sb[:, 0:LR], in_=kv_gath[0:64, :])
            nc.sync.dma_start(out=kt_sb[:, LR:S], in_=kv_gath[128:192, :])
            nc.sync.dma_start(out=vt_sb[:, 0:LR], in_=kv_gath[64:128, :])
            nc.sync.dma_start(out=vt_sb[:, LR:S], in_=kv_gath[192:256, :])

            # ---- V' = [V | 1] per gathered chunk ----
            NCH = S // KC  # 32
            vp_sb = gath_pool.tile([128, NCH, H + 1], BF, tag="vp")
            nc.gpsimd.memset(vp_sb[:, :, H], 1.0)
            for grp in range(NCH // 4):
                vt_ps = pvt_pool.tile([128, 4 * H], BF, tag="vtps")
                for u in range(4):
                    ch = grp * 4 + u
                    nc.tensor.transpose(vt_ps[:, u * H:(u + 1) * H],
                                        vt_sb[:, ch * KC:(ch + 1) * KC],
                                        idb_sb[0:64, 0:64])
                nc.vector.tensor_copy(
                    vp_sb[:, grp * 4:(grp + 1) * 4, 0:H],
                    vt_ps.rearrange("p (u h) -> p u h", u=4))

            # ---- attention ----
            for j in range(NQT):
                qs = slice(j * QT, (j + 1) * QT)
                U = 8 * j + 8
                outp = pout_pool.tile([65, QT], F32, tag="outp")
                for c in range(U):
                    kc = _gid(c)
                    sc_ps = psc_pool.tile([128, QT], F32, tag="scps")
                    nc.tensor.matmul(sc_ps[:],
                                     kt_sb[:, kc * KC:(kc + 1) * KC],
                                     qqt_sb[0:64, qs],
                                     start=True, stop=True)
                    p_sb = work_pool.tile([128, QT], BF, tag="p")
                    nc.scalar.activation(p_sb[:], sc_ps[:],
                                         mybir.ActivationFunctionType.Exp,
                                         scale=float(H) ** -0.5)
                    mi = c - 8 * j
                    if mi >= 0:
                        nc.vector.tensor_mul(p_sb[:], p_sb[:],
                                             masks_sb[:, mi, :])
                    nc.tensor.matmul(outp[:],
                                     vp_sb[:, kc, :],
                                     p_sb[:],
                                     start=(c == 0), stop=(c == U - 1))
                # normalize + emit
                ot_sb = out_pool.tile([65, QT], F32, tag="ot")
                nc.vector.tensor_copy(ot_sb[:], outp[:])
                for jj in range(4):
                    o_ps = pops_pool.tile([128, 65], F32, tag="ops")
                    nc.tensor.transpose(o_ps[:],
                                        ot_sb[:, jj * 128:(jj + 1) * 128],
                                        idf_sb[0:65, 0:65])
                    r_sb = out_pool.tile([128, 1], F32, tag="r")
                    nc.vector.reciprocal(r_sb[:], o_ps[:, 64:65])
                    o_sb = out_pool.tile([128, H], F32, tag="o")
                    nc.vector.tensor_scalar_mul(o_sb[:], o_ps[:, 0:H], r_sb[:])
                    r0 = j * QT + jj * 128
                    nc.sync.dma_start(out=out_ext[r0:r0 + 128, :], in_=o_sb[:])

    nc.compile()
    _NC_CACHE["nc"] = nc
    return nc


def _make_masks(parity):
    kk = np.arange(KC)[:, None]
    qq = np.arange(QT)[None, :]
    diags = [(qq >= kk + 128 * i).astype(np.float32) for i in range(4)]
    ones = np.ones((KC, QT), np.float32)
    zeros = np.zeros((KC, QT), np.float32)
    if parity == 0:
        pats = diags + [zeros] * 4
    else:
        pats = [ones] * 4 + diags
    return np.stack(pats).astype(ml_dtypes.bfloat16)


def _prep_inputs(x, Wq, Wk, Wv):
    wkv = np.ascontiguousarray(
        np.concatenate([Wk.T, Wv.T], axis=1), dtype=np.float32)
    wqq = np.ascontiguousarray(
        np.concatenate([Wq.T, Wq.T], axis=1), dtype=np.float32)
    idb = np.eye(128, dtype=ml_dtypes.bfloat16)
    idf = np.eye(128, dtype=np.float32)
    masks = [_make_masks(0), _make_masks(1)]
    in_maps = []
    for c in range(NCORES):
        b, par = c // 2, c % 2
        blocks = [2 * i + par for i in range(4)]
        rows = np.concatenate([x[b, blk * 512:(blk + 1) * 512] for blk in blocks])
        xt = np.ascontiguousarray(rows.T, dtype=np.float32)
        in_maps.append({
            "xt": xt,
            "wkv": wkv,
            "wqq": wqq,
            "masks": masks[par],
            "idb": idb,
            "idf": idf,
        })
    return in_maps


def _assemble(results):
    out = np.empty((B, S, H), np.float32)
    for c in range(NCORES):
        b, par = c // 2, c % 2
        r = results[c]["out"]
        for i in range(4):
            blk = 2 * i + par
            out[b, blk * 512:(blk + 1) * 512] = r[i * 512:(i + 1) * 512]
    return out


def run(x, Wq, Wk, Wv, trace=False):
    nc = build_nc()
    in_maps = _prep_inputs(x, Wq, Wk, Wv)
    res = run_bass_kernel_spmd(nc, in_maps, core_ids=list(range(NCORES)),
                               trace=trace)
    return _assemble(res.results), res


def kernel(x, Wq, Wk, Wv):
    x = np.asarray(x, dtype=np.float32)
    out, _ = run(x, np.asarray(Wq, np.float32), np.asarray(Wk, np.float32),
                 np.asarray(Wv, np.float32))
    return out


# revision 7
# speedup vs baseline: 1.4727x; 1.4727x over previous
"""Distributed Trainium2 Bass kernel for a single causal attention head.

Problem: x [4, 4096, 1024] f32, Wq/Wk/Wv [64, 1024] f32.
  q/k/v = x @ W.T ; scores = q k^T / sqrt(64); causal softmax; out = attn @ v.
Output: [4, 4096, 64] f32.

Sharding (8 cores): one batch per core-pair; within a pair the sequence is
interleaved at 512-row granularity (even core: 512-blocks {0,2,4,6}, odd:
{1,3,5,7}).  Each core computes Q/K/V for its own 2048 rows from a
host-pre-transposed bf16 x^T shard, the pair exchanges [K^T;V^T] via two
pipelined pairwise AllGathers (half h issued right after KV projection
tiles 2h, 2h+1 finish), then each core runs block-causal attention for its
queries over all 4096 keys.

SPMD: all 8 cores execute one graph.  The causal structure differs between
pair members, so the schedule processes, for q-subtile j (512 queries),
the union count U_j = 8j+8 key chunks (128 keys each); the last 8 slots of
each subtile multiply by a per-core mask *datum* (even core: diagonal
masks then zeros; odd core: ones then diagonal masks).  80 chunk slots per
core vs 72 useful -- 11% padding buys graph uniformity.

Attention slots run in row-packed pairs on the PE: chunk 2i contracts on
array rows 0-63 (K^T copy A), chunk 2i+1 on rows 64-127 (copy B) against
the duplicated Q^T, concurrently via tile_position -- then one 1024-wide
exp (ScalarE, 1/8 scale fused; no max-subtraction needed, scores~N(0,1))
and two PV matmuls with stationary V' = [V | 1 | 0pad] whose ones column
yields the softmax denominators in out^T row 64.
"""

import numpy as np
import ml_dtypes

import concourse.bacc as bacc
import concourse.mybir as mybir
import concourse.tile as tile
from concourse.bass_utils import run_bass_kernel_spmd

B, S, E, H = 4, 4096, 1024, 64
NCORES = 8
LR = 2048            # local rows per core
QT = 512             # q tile width
KC = 128             # key chunk
NQT = LR // QT       # 4 q-subtiles per core
NEC = E // 128       # 8 e-chunks
HALF = LR // 2       # kvt columns per collective half
GROUPS = [[0, 1], [2, 3], [4, 5], [6, 7]]

BF = mybir.dt.bfloat16
F32 = mybir.dt.float32

_NC_CACHE = {}


def _gid(c):
    """Global 128-key chunk c (0..31) -> gathered chunk index."""
    blk = c // 4
    w = c % 4
    if blk % 2 == 0:
        return (blk // 2) * 4 + w
    return 16 + (blk // 2) * 4 + w


def _slot_addr(c):
    """Slot (global chunk) c -> (half, colblock, rowhalf, vidx).

    kt2[half] is [128, 8*KC]: col-blocks 0..3 hold the even-core gathered
    chunk pairs of that half, 4..7 the odd-core pairs; rows 0:64 = even
    gathered chunk of the pair, rows 64:128 = odd.
    vp[half] is [128, 16, 128]: index 0..7 even-core chunks, 8..15 odd's.
    """
    g = _gid(c)
    half = 1 if (g % 16) >= 8 else 0
    side = 0 if g < 16 else 1
    within = g % 8
    colblock = side * 4 + within // 2
    rowhalf = 64 * (g % 2)
    vidx = side * 8 + within
    return half, colblock, rowhalf, vidx


def build_nc():
    if "nc" in _NC_CACHE:
        return _NC_CACHE["nc"]

    nc = bacc.Bacc("TRN2", target_bir_lowering=False, debug=False,
                   num_devices=NCORES)

    xt_ext = nc.dram_tensor("xt", [E, LR], BF, kind="ExternalInput")
    wkv_ext = nc.dram_tensor("wkv", [E, 128], BF, kind="ExternalInput")
    wqq_ext = nc.dram_tensor("wqq", [E, 128], BF, kind="ExternalInput")
    masks_ext = nc.dram_tensor("masks", [8, 128, QT], BF, kind="ExternalInput")
    idb_ext = nc.dram_tensor("idb", [128, 128], BF, kind="ExternalInput")
    idf_ext = nc.dram_tensor("idf", [128, 128], F32, kind="ExternalInput")
    out_ext = nc.dram_tensor("out", [LR, H], F32, kind="ExternalOutput")

    with tile.TileContext(nc) as tc:
        with (
            tc.tile_pool(name="const", bufs=1) as const_pool,
            tc.tile_pool(name="proj", bufs=1) as proj_pool,
            tc.tile_pool(name="gath", bufs=1) as gath_pool,
            tc.tile_pool(name="work", bufs=4) as work_pool,
            tc.tile_pool(name="outw", bufs=3) as out_pool,
            tc.tile_pool(name="dram", bufs=1, space="DRAM") as dram_pool,
        ):
            # ---- constants ----
            wkv_sb = const_pool.tile([128, NEC, 128], BF, tag="wkv")
            nc.sync.dma_start(out=wkv_sb[:], in_=wkv_ext.ap().rearrange(
                "(c p) m -> p c m", p=128))
            wqq_sb = const_pool.tile([128, NEC, 128], BF, tag="wqq")
            nc.sync.dma_start(out=wqq_sb[:], in_=wqq_ext.ap().rearrange(
                "(c p) m -> p c m", p=128))
            masks_sb = const_pool.tile([128, 8, QT], BF, tag="masks")
            nc.sync.dma_start(out=masks_sb[:], in_=masks_ext.ap().rearrange(
                "m p q -> p m q"))
            idb_sb = const_pool.tile([128, 128], BF, tag="idb")
            nc.sync.dma_start(out=idb_sb[:], in_=idb_ext[:])
            idf_sb = const_pool.tile([128, 128], F32, tag="idf")
            nc.sync.dma_start(out=idf_sb[:], in_=idf_ext[:])

            kvt_sb = proj_pool.tile([128, LR], BF, tag="kvt")   # [K^T; V^T]
            qqt_sb = proj_pool.tile([128, LR], BF, tag="qqt")   # [Q^T; Q^T]
            kv_bounce = [dram_pool.tile([128, HALF], BF, tag=f"kvb{h}",
                                        name=f"kvb{h}") for h in range(2)]
            kv_gath = [dram_pool.tile([256, HALF], BF, tag=f"kvg{h}",
                                      name=f"kvg{h}") for h in range(2)]

            # ---- x^T loads + projections, split AllGather ----
            with (
                tc.tile_pool(name="xt", bufs=NQT * NEC) as xt_pool,
                tc.tile_pool(name="pkvq", bufs=2, space="PSUM") as pkvq_pool,
            ):
                xt_sb = [[None] * NEC for _ in range(NQT)]
                for qt in range(NQT):
                    qs = slice(qt * QT, (qt + 1) * QT)
                    for ec in range(NEC):
                        t = xt_pool.tile([128, QT], BF, tag="xt")
                        nc.sync.dma_start(
                            out=t[:], in_=xt_ext[ec * 128:(ec + 1) * 128, qs])
                        xt_sb[qt][ec] = t
                    kv_ps = pkvq_pool.tile([128, QT], F32, tag="kvps")
                    for ec in range(NEC):
                        nc.tensor.matmul(kv_ps[:], wkv_sb[:, ec, :],
                                         xt_sb[qt][ec][:],
                                         start=(ec == 0), stop=(ec == NEC - 1))
                    nc.scalar.copy(kvt_sb[:, qs], kv_ps[:])
                    if qt % 2 == 1:
                        h = qt // 2
                        cs = slice(h * HALF, (h + 1) * HALF)
                        nc.sync.dma_start(out=kv_bounce[h][:],
                                          in_=kvt_sb[:, cs])
                        nc.gpsimd.collective_compute(
                            "AllGather",
                            mybir.AluOpType.bypass,
                            replica_groups=GROUPS,
                            ins=[kv_bounce[h][:]],
                            outs=[kv_gath[h][:]],
                        )
                # Q projections overlap the collectives
                for qt in range(NQT):
                    qs = slice(qt * QT, (qt + 1) * QT)
                    qq_ps = pkvq_pool.tile([128, QT], F32, tag="qqps")
                    for ec in range(NEC):
                        nc.tensor.matmul(qq_ps[:], wqq_sb[:, ec, :],
                                         xt_sb[qt][ec][:],
                                         start=(ec == 0), stop=(ec == NEC - 1))
                    nc.vector.tensor_copy(qqt_sb[:, qs], qq_ps[:])

            # ---- gather-back: kt2 pair layout + V' ----
            kt2 = []
            vp = []
            vt_sb = []
            for hf in range(2):
                kt = gath_pool.tile([128, 8 * KC], BF, tag=f"kt{hf}", name=f"kt{hf}")
                kt2.append(kt)
                vt = gath_pool.tile([64, 16 * KC], BF, tag=f"vt{hf}", name=f"vt{hf}")
                vt_sb.append(vt)
                for side, rbase in ((0, 0), (1, 128)):
                    ksrc = kv_gath[hf][rbase:rbase + 64, :].rearrange(
                        "p (u two w) -> p u two w", two=2, w=KC)
                    cb = slice(side * 4 * KC, (side + 1) * 4 * KC)
                    nc.sync.dma_start(
                        out=kt[0:64, cb].rearrange("p (u w) -> p u w", w=KC),
                        in_=ksrc[:, :, 0, :])
                    nc.sync.dma_start(
                        out=kt[64:128, cb].rearrange("p (u w) -> p u w", w=KC),
                        in_=ksrc[:, :, 1, :])
                    nc.sync.dma_start(
                        out=vt[:, side * 8 * KC:(side + 1) * 8 * KC],
                        in_=kv_gath[hf][rbase + 64:rbase + 128, :])

            with tc.tile_pool(name="pvt", bufs=2, space="PSUM") as pvt_pool:
                for hf in range(2):
                    vpt = gath_pool.tile([128, 16, 128], BF, tag=f"vp{hf}", name=f"vp{hf}")
                    vp.append(vpt)
                    nc.gpsimd.memset(vpt[:, :, H], 1.0)
                    nc.gpsimd.memset(vpt[:, :, H + 1:128], 0.0)
                    for grp in range(4):
                        vt_ps = pvt_pool.tile([128, 4 * H], BF, tag="vtps")
                        for u in range(4):
                            ch = grp * 4 + u
                            nc.tensor.transpose(
                                vt_ps[:, u * H:(u + 1) * H],
                                vt_sb[hf][:, ch * KC:(ch + 1) * KC],
                                idb_sb[0:64, 0:64])
                        nc.vector.tensor_copy(
                            vp[hf][:, grp * 4:(grp + 1) * 4, 0:H],
                            vt_ps.rearrange("p (u h) -> p u h", u=4))

            # ---- attention ----
            with (
                tc.tile_pool(name="psc", bufs=2, space="PSUM") as psc_pool,
                tc.tile_pool(name="pout", bufs=2, space="PSUM") as pout_pool,
                tc.tile_pool(name="pops", bufs=1, space="PSUM") as pops_pool,
            ):
                for j in range(NQT):
                    qs = slice(j * QT, (j + 1) * QT)
                    U = 8 * j + 8
                    outp = pout_pool.tile([128, QT], F32, tag="outp")
                    for i in range(U // 2):
                        sc = psc_pool.tile([128, 2 * QT], F32, tag="scps")
                        for t in range(2):
                            c = 2 * i + t
                            hf, cb, rh, _ = _slot_addr(c)
                            nc.tensor.matmul(
                                sc[:, t * QT:(t + 1) * QT],
                                kt2[hf][rh:rh + 64, cb * KC:(cb + 1) * KC],
                                qqt_sb[rh:rh + 64, qs],
                                start=True, stop=True,
                                tile_position=(rh, 0))
                        p_sb = work_pool.tile([128, 2 * QT], BF, tag="p")
                        nc.scalar.activation(p_sb[:], sc[:],
                                             mybir.ActivationFunctionType.Exp,
                                             scale=float(H) ** -0.5)
                        for t in range(2):
                            c = 2 * i + t
                            mi = c - 8 * j
                            if mi >= 0:
                                nc.vector.tensor_mul(
                                    p_sb[:, t * QT:(t + 1) * QT],
                                    p_sb[:, t * QT:(t + 1) * QT],
                                    masks_sb[:, mi, :])
                        for t in range(2):
                            c = 2 * i + t
                            hf, _, _, vidx = _slot_addr(c)
                            nc.tensor.matmul(
                                outp[:], vp[hf][:, vidx, :],
                                p_sb[:, t * QT:(t + 1) * QT],
                                start=(c == 0), stop=(c == U - 1))
                    # normalize + emit
                    ot_sb = out_pool.tile([65, QT], F32, tag="ot")
                    nc.vector.tensor_copy(ot_sb[:], outp[0:65, :])
                    for jj in range(4):
                        o_ps = pops_pool.tile([128, 65], F32, tag="ops")
                        nc.tensor.transpose(o_ps[:],
                                            ot_sb[:, jj * 128:(jj + 1) * 128],
                                            idf_sb[0:65, 0:65])
                        r_sb = out_pool.tile([128, 1], F32, tag="r")
                        nc.vector.reciprocal(r_sb[:], o_ps[:, 64:65])
                        o_sb = out_pool.tile([128, H], F32, tag="o")
                        nc.vector.tensor_scalar_mul(o_sb[:], o_ps[:, 0:H],
                                                    r_sb[:])
                        r0 = j * QT + jj * 128
                        nc.sync.dma_start(out=out_ext[r0:r0 + 128, :],
                                          in_=o_sb[:])

    nc.compile()
    _NC_CACHE["nc"] = nc
    return nc


def _make_masks(parity):
    kk = np.arange(KC)[:, None]
    qq = np.arange(QT)[None, :]
    diags = [(qq >= kk + 128 * i).astype(np.float32) for i in range(4)]
    ones = np.ones((KC, QT), np.float32)
    zeros = np.zeros((KC, QT), np.float32)
    if parity == 0:
        pats = diags + [zeros] * 4
    else:
        pats = [ones] * 4 + diags
    return np.stack(pats).astype(ml_dtypes.bfloat16)


def _prep_inputs(x, Wq, Wk, Wv):
    wkv = np.concatenate([Wk.T, Wv.T], axis=1).astype(ml_dtypes.bfloat16)
    wqq = np.concatenate([Wq.T, Wq.T], axis=1).astype(ml_dtypes.bfloat16)
    idb = np.eye(128, dtype=ml_dtypes.bfloat16)
    idf = np.eye(128, dtype=np.float32)
    masks = [_make_masks(0), _make_masks(1)]
    in_maps = []
    for c in range(NCORES):
        b, par = c // 2, c % 2
        blocks = [2 * i + par for i in range(4)]
        rows = np.concatenate([x[b, blk * 512:(blk + 1) * 512] for blk in blocks])
        xt = np.ascontiguousarray(rows.T).astype(ml_dtypes.bfloat16)
        in_maps.append({
            "xt": xt,
            "wkv": wkv,
            "wqq": wqq,
            "masks": masks[par],
            "idb": idb,
            "idf": idf,
        })
    return in_maps


def _assemble(results):
    out = np.empty((B, S, H), np.float32)
    for c in range(NCORES):
        b, par = c // 2, c % 2
        r = results[c]["out"]
        for i in range(4):
            blk = 2 * i + par
            out[b, blk * 512:(blk + 1) * 512] = r[i * 512:(i + 1) * 512]
    return out


def run(x, Wq, Wk, Wv, trace=False):
    nc = build_nc()
    in_maps = _prep_inputs(x, Wq, Wk, Wv)
    res = run_bass_kernel_spmd(nc, in_maps, core_ids=list(range(NCORES)),
                               trace=trace)
    return _assemble(res.results), res


def kernel(x, Wq, Wk, Wv):
    x = np.asarray(x, dtype=np.float32)
    out, _ = run(x, np.asarray(Wq, np.float32), np.asarray(Wk, np.float32),
                 np.asarray(Wv, np.float32))
    return out
